# revision 5
# baseline (speedup 1.0000x reference)
"""Haar 2D DWT (pywt 'haar') Trainium2 Bass kernel — bf16 pipeline.

Full input x: [16, 64, 256, 256] f32.
Output: [16, 256, 128, 128] f32 = concat(ll, lh, hl, hh) on channel axis.

Sharding: pure data-parallel over batch (16 -> 2 per core x 8 cores).

The op is memory-bound: per core 33.55 MB in + half-sized bf16 out
(16.78 MB) vs the ~428 GB/s 16-SDMA-engine ceiling. The device computes
and stores bf16 (rel-err tolerance is 2e-2; this path lands ~6.8e-3) and
the host widens to f32.

Per-core layout: partition p = image index (b*C + c) — 128 images per
core, c-major so every DMA's outer AP dim has count 64 and fans across
all 16 SDMA engines. Work proceeds in bands of R row-pairs:

  in-DMA   f32 band (sync/SP HWDGE ring)
  ACT      deinterleave even/odd columns + x0.5 + f32->bf16 convert
           (stride-2 reads run full rate on ACT, and this makes every
            DVE operand packed bf16 -> 2x DVE mode; f32 TENSOR_TENSOR
            runs 1 elem/cycle/lane, packed bf16 runs 2)
  DVE      row butterfly on packed bf16 (se,de,so,do)
  DVE      col butterfly on packed bf16 (ll,lh,hl,hh)
  out-DMA  4 quadrants bf16; out queues are descriptor-rate-bound, so
           outputs are grouped across bands for 8KB descriptors and
           split over the scalar HWDGE + gpsimd SWDGE rings; the last
           groups also use the by-then-idle sync ring.

Band schedule: uniform R=8 (16 bands) with group sizes 1,1,2,4,4,2,1,1 —
small ungrouped edge bands start the out stream early and drain the tail
fast; 4-band middle groups give 8KB out descriptors.

in_bufs=6 (96KB/partition of in-band double-buffering): the in-DMA queue
runs up to 6 bands ahead of ACT, so the DMA bus stays saturated
(~420-433 GB/s measured) through out-stream gaps instead of the in-queue
stalling on tile recycling. This took exec from ~157us to ~131us; the
remaining time is ~7us fixed preamble + 119.7us bus-saturated stream +
~3us tail drain/epilogue. (Deeper in_bufs=7, mid/sd/out buffer bumps,
and edge-band shaping were all tried: no further gain, floor ~131.3us.)
"""

import numpy as np

N_CORES = 8
FULL_B, C, H, W = 16, 64, 256, 256


def _build_bass(B=2, Cc=64, Hh=256, Ww=256, in_bufs=6, mid_bufs=2,
                out_bufs=2, sd_bufs=1, in_ring="sync",
                bands=(8,) * 16, groups=(1, 1, 2, 4, 4, 2, 1, 1),
                out_rings=("scalar", "gpsimd", "scalar", "gpsimd"),
                tail_sync_groups=2, prewarm=0, jump=None):
    import concourse.bacc as bacc
    import concourse.mybir as mybir
    from concourse.tile import TileContext

    P = B * Cc
    HP = Hh // 2          # row pairs per image
    Wh = Ww // 2
    f32 = mybir.dt.float32
    bf16 = mybir.dt.bfloat16

    nc = bacc.Bacc("TRN2", target_bir_lowering=False, debug=False)
    x = nc.dram_tensor("x", [B, Cc, Hh, Ww], f32, kind="ExternalInput").ap()
    y = nc.dram_tensor("y", [B, 4 * Cc, HP, Wh], bf16, kind="ExternalOutput").ap()

    xi = x.rearrange("b c h w -> c b (h w)")
    yo = y.rearrange("b (q c) h w -> q c b (h w)", q=4)

    rings = {"sync": nc.sync, "scalar": nc.scalar, "gpsimd": nc.gpsimd}
    bands = list(bands)
    groups = list(groups)
    assert sum(bands) == HP, bands
    assert sum(groups) == len(bands), (groups, bands)
    g_of_band = []
    for gi, n in enumerate(groups):
        for pos in range(n):
            g_of_band.append((gi, pos, n))

    with TileContext(nc) as tc:
        with tc.tile_pool(name="pool", bufs=2) as pool:
            if prewarm:
                scratch = nc.dram_tensor("warm_scratch", [P, 8], f32,
                                         kind="Internal").ap()
                warm_t = pool.tile([P, 8], f32, tag="warm", bufs=1)
                rings[in_ring].dma_start(out=warm_t[:], in_=xi[:, :, 0:8])
                for wi, wr in enumerate(("scalar", "gpsimd")):
                    rings[wr].dma_start(out=scratch[:, wi * 4:wi * 4 + 4],
                                        in_=warm_t[:, wi * 4:wi * 4 + 4])
            r0s = [sum(bands[:i]) for i in range(len(bands))]
            o_ts = None
            for bi, (r0, R_) in enumerate(zip(r0s, bands)):
                gi, pos, glen = g_of_band[bi]
                n_rows = 2 * R_
                in_t = pool.tile([P, n_rows * Ww], f32, name="in_t", tag="in",
                                 bufs=in_bufs)
                rings[in_ring].dma_start(
                    out=in_t[:], in_=xi[:, :, r0 * 2 * Ww:(r0 + R_) * 2 * Ww])

                # iv[p, rr, w, t]: row rr, col pair w, even/odd col t
                iv = in_t[:].rearrange("p (rr w t) -> p rr w t", rr=n_rows, t=2)
                xe_t = pool.tile([P, n_rows * Wh], bf16, tag="xe", bufs=mid_bufs)
                xo_t = pool.tile([P, n_rows * Wh], bf16, tag="xo", bufs=mid_bufs)
                xev = xe_t[:].rearrange("p (rr j) -> p rr j", rr=n_rows)
                xov = xo_t[:].rearrange("p (rr j) -> p rr j", rr=n_rows)
                nc.scalar.mul(xev[:, :, :], iv[:, :, :, 0], 0.5)
                nc.scalar.mul(xov[:, :, :], iv[:, :, :, 1], 0.5)

                # row butterfly: pair rows (2i, 2i+1), packed bf16
                xep = xe_t[:].rearrange("p (r t j) -> p r t j", t=2, j=Wh)
                xop = xo_t[:].rearrange("p (r t j) -> p r t j", t=2, j=Wh)
                sdt = [pool.tile([P, R_ * Wh], bf16, name=f"sd{i}",
                                 tag=f"sd{i}", bufs=sd_bufs)
                       for i in range(4)]
                se, de, so, do_ = [t[:].rearrange("p (r j) -> p r j", j=Wh)
                                   for t in sdt]
                nc.vector.tensor_add(out=se, in0=xep[:, :, 0, :], in1=xep[:, :, 1, :])
                nc.vector.tensor_sub(out=de, in0=xep[:, :, 0, :], in1=xep[:, :, 1, :])
                nc.vector.tensor_add(out=so, in0=xop[:, :, 0, :], in1=xop[:, :, 1, :])
                nc.vector.tensor_sub(out=do_, in0=xop[:, :, 0, :], in1=xop[:, :, 1, :])

                # col butterfly -> grouped quadrant outputs
                if pos == 0:
                    gsz = sum(bands[bi:bi + glen]) * Wh
                    gr0 = r0
                    goff = 0
                    o_ts = [pool.tile([P, gsz], bf16, name=f"o{q}",
                                      tag=f"o{q}", bufs=out_bufs)
                            for q in range(4)]
                sl = slice(goff, goff + R_ * Wh)
                ll, lh, hl, hh = [o[:, sl] for o in o_ts]
                nc.vector.tensor_add(out=ll, in0=sdt[0][:], in1=sdt[2][:])
                nc.vector.tensor_add(out=lh, in0=sdt[1][:], in1=sdt[3][:])
                nc.vector.tensor_sub(out=hl, in0=sdt[0][:], in1=sdt[2][:])
                nc.vector.tensor_sub(out=hh, in0=sdt[1][:], in1=sdt[3][:])
                goff += R_ * Wh
                if pos == glen - 1:
                    Rg = goff // Wh
                    # tail groups: the in-queue (sync/SP HWDGE) is drained
                    # by then — spread out-DMAs over it too
                    qrings = (["sync", "scalar", "gpsimd", "sync"]
                              if gi >= len(groups) - tail_sync_groups
                              else list(out_rings))
                    for q in range(4):
                        rings[qrings[q]].dma_start(
                            out=yo[q][:, :, gr0 * Wh:(gr0 + Rg) * Wh],
                            in_=o_ts[q][:, :Rg * Wh])
    nc.compile()
    return nc


def kernel(x: np.ndarray) -> np.ndarray:
    from concourse.bass_utils import run_bass_kernel_spmd

    x = np.ascontiguousarray(np.asarray(x, dtype=np.float32))
    assert x.shape == (FULL_B, C, H, W), x.shape
    nc = _build_bass()
    shards = np.split(x, N_CORES, axis=0)
    in_maps = [{"x": s} for s in shards]
    res = run_bass_kernel_spmd(nc, in_maps, list(range(N_CORES)))
    return np.concatenate(
        [np.asarray(r["y"]).astype(np.float32) for r in res.results], axis=0)



# revision 6
# speedup vs baseline: 1.1482x; 1.1482x over previous
"""Haar 2D DWT (pywt 'haar') Trainium2 Bass kernel — bf16 pipeline.

Full input x: [16, 64, 256, 256] f32.
Output: [16, 256, 128, 128] f32 = concat(ll, lh, hl, hh) on channel axis.

Sharding: pure data-parallel over batch (16 -> 2 per core x 8 cores).

The op is memory-bound: per core 33.55 MB in + half-sized bf16 out
(16.78 MB) vs the ~428 GB/s 16-SDMA-engine ceiling. The device computes
and stores bf16 (rel-err tolerance is 2e-2; this path lands ~6.8e-3) and
the host widens to f32.

Per-core layout: partition p = image index (b*C + c) — 128 images per
core, c-major so every DMA's outer AP dim has count 64 and fans across
all 16 SDMA engines. Work proceeds in bands of R row-pairs:

  in-DMA   f32 band (sync/SP HWDGE ring)
  ACT      deinterleave even/odd columns + x0.5 + f32->bf16 convert
           (stride-2 reads run full rate on ACT, and this makes every
            DVE operand packed bf16 -> 2x DVE mode; f32 TENSOR_TENSOR
            runs 1 elem/cycle/lane, packed bf16 runs 2)
  DVE      row butterfly on packed bf16 (se,de,so,do)
  DVE      col butterfly on packed bf16 (ll,lh,hl,hh)
  out-DMA  4 quadrants bf16; out queues are descriptor-rate-bound, so
           outputs are grouped across bands for 8KB descriptors and
           split over the scalar HWDGE + gpsimd SWDGE rings; the last
           groups also use the by-then-idle sync ring.

Band schedule: uniform R=8 (16 bands) with group sizes 1,1,2,4,4,2,1,1 —
small ungrouped edge bands start the out stream early and drain the tail
fast; 4-band middle groups give 8KB out descriptors.

in_bufs=6 (96KB/partition of in-band double-buffering): the in-DMA queue
runs up to 6 bands ahead of ACT, so the DMA bus stays saturated
(~420-433 GB/s measured) through out-stream gaps instead of the in-queue
stalling on tile recycling. This took exec from ~157us to ~131us; the
remaining time is ~7us fixed preamble + 119.7us bus-saturated stream +
~3us tail drain/epilogue. (Deeper in_bufs=7, mid/sd/out buffer bumps,
and edge-band shaping were all tried: no further gain, floor ~131.3us.)
"""

import numpy as np

N_CORES = 8
FULL_B, C, H, W = 16, 64, 256, 256


def _build_bass(B=2, Cc=64, Hh=256, Ww=256, in_bufs=6, mid_bufs=2,
                out_bufs=2, sd_bufs=1, in_ring="sync",
                bands=(8,) * 16, groups=(1, 1, 2, 4, 4, 2, 1, 1),
                out_rings=("scalar", "gpsimd", "scalar", "gpsimd"),
                tail_sync_groups=2, prewarm=0, jump=None):
    import concourse.bacc as bacc
    import concourse.mybir as mybir
    from concourse.tile import TileContext

    P = B * Cc
    HP = Hh // 2          # row pairs per image
    Wh = Ww // 2
    f32 = mybir.dt.float32
    bf16 = mybir.dt.bfloat16

    nc = bacc.Bacc("TRN2", target_bir_lowering=False, debug=False)
    x = nc.dram_tensor("x", [B, Cc, Hh, Ww], f32, kind="ExternalInput").ap()
    y = nc.dram_tensor("y", [B, 4 * Cc, HP, Wh], bf16, kind="ExternalOutput").ap()

    xi = x.rearrange("b c h w -> c b (h w)")
    yo = y.rearrange("b (q c) h w -> q c b (h w)", q=4)

    rings = {"sync": nc.sync, "scalar": nc.scalar, "gpsimd": nc.gpsimd}
    bands = list(bands)
    groups = list(groups)
    assert sum(bands) == HP, bands
    assert sum(groups) == len(bands), (groups, bands)
    g_of_band = []
    for gi, n in enumerate(groups):
        for pos in range(n):
            g_of_band.append((gi, pos, n))

    with TileContext(nc) as tc:
        with tc.tile_pool(name="pool", bufs=2) as pool:
            if prewarm:
                scratch = nc.dram_tensor("warm_scratch", [P, 8], f32,
                                         kind="Internal").ap()
                warm_t = pool.tile([P, 8], f32, tag="warm", bufs=1)
                rings[in_ring].dma_start(out=warm_t[:], in_=xi[:, :, 0:8])
                for wi, wr in enumerate(("scalar", "gpsimd")):
                    rings[wr].dma_start(out=scratch[:, wi * 4:wi * 4 + 4],
                                        in_=warm_t[:, wi * 4:wi * 4 + 4])
            r0s = [sum(bands[:i]) for i in range(len(bands))]
            o_ts = None
            jump_tile = None
            if jump is not None:
                # Pre-feed band `jump` via the scalar HWDGE ring at program
                # start: its descriptors sit ahead of all out-groups in that
                # queue and land mid-stream, so the sync queue's LAST item
                # (the final band) can be tiny -> short post-in-end tail.
                r0j, Rj = r0s[jump], bands[jump]
                jump_tile = pool.tile([P, 2 * Rj * Ww], f32, name="in_jump",
                                      tag="in_jump", bufs=1)
                rings["scalar"].dma_start(
                    out=jump_tile[:],
                    in_=xi[:, :, r0j * 2 * Ww:(r0j + Rj) * 2 * Ww])
            for bi, (r0, R_) in enumerate(zip(r0s, bands)):
                gi, pos, glen = g_of_band[bi]
                n_rows = 2 * R_
                if bi == jump:
                    in_t = jump_tile
                else:
                    in_t = pool.tile([P, n_rows * Ww], f32, name="in_t",
                                     tag="in", bufs=in_bufs)
                    rings[in_ring].dma_start(
                        out=in_t[:],
                        in_=xi[:, :, r0 * 2 * Ww:(r0 + R_) * 2 * Ww])

                # iv[p, rr, w, t]: row rr, col pair w, even/odd col t
                iv = in_t[:].rearrange("p (rr w t) -> p rr w t", rr=n_rows, t=2)
                xe_t = pool.tile([P, n_rows * Wh], bf16, tag="xe", bufs=mid_bufs)
                xo_t = pool.tile([P, n_rows * Wh], bf16, tag="xo", bufs=mid_bufs)
                xev = xe_t[:].rearrange("p (rr j) -> p rr j", rr=n_rows)
                xov = xo_t[:].rearrange("p (rr j) -> p rr j", rr=n_rows)
                nc.scalar.mul(xev[:, :, :], iv[:, :, :, 0], 0.5)
                nc.scalar.mul(xov[:, :, :], iv[:, :, :, 1], 0.5)

                # row butterfly: pair rows (2i, 2i+1), packed bf16
                xep = xe_t[:].rearrange("p (r t j) -> p r t j", t=2, j=Wh)
                xop = xo_t[:].rearrange("p (r t j) -> p r t j", t=2, j=Wh)
                sdt = [pool.tile([P, R_ * Wh], bf16, name=f"sd{i}",
                                 tag=f"sd{i}", bufs=sd_bufs)
                       for i in range(4)]
                se, de, so, do_ = [t[:].rearrange("p (r j) -> p r j", j=Wh)
                                   for t in sdt]
                nc.vector.tensor_add(out=se, in0=xep[:, :, 0, :], in1=xep[:, :, 1, :])
                nc.vector.tensor_sub(out=de, in0=xep[:, :, 0, :], in1=xep[:, :, 1, :])
                nc.vector.tensor_add(out=so, in0=xop[:, :, 0, :], in1=xop[:, :, 1, :])
                nc.vector.tensor_sub(out=do_, in0=xop[:, :, 0, :], in1=xop[:, :, 1, :])

                # col butterfly -> grouped quadrant outputs
                if pos == 0:
                    gsz = sum(bands[bi:bi + glen]) * Wh
                    gr0 = r0
                    goff = 0
                    o_ts = [pool.tile([P, gsz], bf16, name=f"o{q}",
                                      tag=f"o{q}", bufs=out_bufs)
                            for q in range(4)]
                sl = slice(goff, goff + R_ * Wh)
                ll, lh, hl, hh = [o[:, sl] for o in o_ts]
                nc.vector.tensor_add(out=ll, in0=sdt[0][:], in1=sdt[2][:])
                nc.vector.tensor_add(out=lh, in0=sdt[1][:], in1=sdt[3][:])
                nc.vector.tensor_sub(out=hl, in0=sdt[0][:], in1=sdt[2][:])
                nc.vector.tensor_sub(out=hh, in0=sdt[1][:], in1=sdt[3][:])
                goff += R_ * Wh
                if pos == glen - 1:
                    Rg = goff // Wh
                    # tail groups: the in-queue (sync/SP HWDGE) is drained
                    # by then — spread out-DMAs over it too
                    qrings = (["sync", "scalar", "gpsimd", "sync"]
                              if gi >= len(groups) - tail_sync_groups
                              else list(out_rings))
                    for q in range(4):
                        rings[qrings[q]].dma_start(
                            out=yo[q][:, :, gr0 * Wh:(gr0 + Rg) * Wh],
                            in_=o_ts[q][:, :Rg * Wh])
    nc.compile()
    return nc


def kernel(x: np.ndarray) -> np.ndarray:
    from concourse.bass_utils import run_bass_kernel_spmd

    x = np.ascontiguousarray(np.asarray(x, dtype=np.float32))
    assert x.shape == (FULL_B, C, H, W), x.shape
    nc = _build_bass()
    shards = np.split(x, N_CORES, axis=0)
    in_maps = [{"x": s} for s in shards]
    res = run_bass_kernel_spmd(nc, in_maps, list(range(N_CORES)))
    return np.concatenate(
        [np.asarray(r["y"]).astype(np.float32) for r in res.results], axis=0)



# revision 9
# speedup vs baseline: 1.3287x; 1.1573x over previous
"""Haar 2D DWT (pywt 'haar') Trainium2 Bass kernel — bf16 pipeline.

Full input x: [16, 64, 256, 256] f32.
Output: [16, 256, 128, 128] f32 = concat(ll, lh, hl, hh) on channel axis.

Sharding: pure data-parallel over batch (16 -> 2 per core x 8 cores).

The op is memory-bound: per core 33.55 MB in + half-sized bf16 out
(16.78 MB) vs the ~428 GB/s 16-SDMA-engine ceiling. The device computes
and stores bf16 (rel-err tolerance is 2e-2; this path lands ~6.8e-3) and
the host widens to f32.

Per-core layout: partition p = image index (b*C + c) — 128 images per
core, c-major so every DMA's outer AP dim has count 64 and fans across
all 16 SDMA engines. Work proceeds in bands of R row-pairs:

  in-DMA   f32 band (sync/SP HWDGE ring)
  ACT      deinterleave even/odd columns + x0.5 + f32->bf16 convert
           (stride-2 reads run full rate on ACT, and this makes every
            DVE operand packed bf16 -> 2x DVE mode; f32 TENSOR_TENSOR
            runs 1 elem/cycle/lane, packed bf16 runs 2)
  DVE      row butterfly on packed bf16 (se,de,so,do)
  DVE      col butterfly on packed bf16 (ll,lh,hl,hh)
  out-DMA  4 quadrants bf16; out queues are descriptor-rate-bound, so
           outputs are grouped across bands for 8KB descriptors and
           split over the scalar HWDGE + gpsimd SWDGE rings; the last
           groups also use the by-then-idle sync ring.

Band schedule: uniform R=8 (16 bands) with group sizes 1,1,2,4,4,2,1,1 —
small ungrouped edge bands start the out stream early and drain the tail
fast; 4-band middle groups give 8KB out descriptors.

in_bufs=6 (96KB/partition of in-band double-buffering): the in-DMA queue
runs up to 6 bands ahead of ACT, so the DMA bus stays saturated
(~420-433 GB/s measured) through out-stream gaps instead of the in-queue
stalling on tile recycling. This took exec from ~157us to ~131us; the
remaining time is ~7us fixed preamble + 119.7us bus-saturated stream +
~3us tail drain/epilogue. (Deeper in_bufs=7, mid/sd/out buffer bumps,
and edge-band shaping were all tried: no further gain, floor ~131.3us.)
"""

import numpy as np

N_CORES = 8
FULL_B, C, H, W = 16, 64, 256, 256


def _build_bass(B=2, Cc=64, Hh=256, Ww=256, in_bufs=6, mid_bufs=2,
                out_bufs=2, sd_bufs=1, in_ring="sync",
                bands=(8,) * 16, groups=(1, 1, 2, 4, 4, 2, 1, 1),
                out_rings=("scalar", "gpsimd", "scalar", "gpsimd"),
                tail_sync_groups=2, prewarm=0, jump=None, jump_emit=3):
    import concourse.bacc as bacc
    import concourse.mybir as mybir
    from concourse.tile import TileContext

    P = B * Cc
    HP = Hh // 2          # row pairs per image
    Wh = Ww // 2
    f32 = mybir.dt.float32
    bf16 = mybir.dt.bfloat16

    nc = bacc.Bacc("TRN2", target_bir_lowering=False, debug=False)
    x = nc.dram_tensor("x", [B, Cc, Hh, Ww], f32, kind="ExternalInput").ap()
    y = nc.dram_tensor("y", [B, 4 * Cc, HP, Wh], bf16, kind="ExternalOutput").ap()

    xi = x.rearrange("b c h w -> c b (h w)")
    yo = y.rearrange("b (q c) h w -> q c b (h w)", q=4)

    rings = {"sync": nc.sync, "scalar": nc.scalar, "gpsimd": nc.gpsimd}
    bands = list(bands)
    groups = list(groups)
    assert sum(bands) == HP, bands
    assert sum(groups) == len(bands), (groups, bands)
    g_of_band = []
    for gi, n in enumerate(groups):
        for pos in range(n):
            g_of_band.append((gi, pos, n))

    with TileContext(nc) as tc:
        with tc.tile_pool(name="pool", bufs=2) as pool:
            if prewarm:
                scratch = nc.dram_tensor("warm_scratch", [P, 8], f32,
                                         kind="Internal").ap()
                warm_t = pool.tile([P, 8], f32, tag="warm", bufs=1)
                rings[in_ring].dma_start(out=warm_t[:], in_=xi[:, :, 0:8])
                for wi, wr in enumerate(("scalar", "gpsimd")):
                    rings[wr].dma_start(out=scratch[:, wi * 4:wi * 4 + 4],
                                        in_=warm_t[:, wi * 4:wi * 4 + 4])
            r0s = [sum(bands[:i]) for i in range(len(bands))]
            o_ts = None
            jump_tile = None
            if jump is not None:
                # Pre-feed band `jump` via the scalar HWDGE ring at program
                # start: its descriptors sit ahead of all out-groups in that
                # queue and land mid-stream, so the sync queue's LAST item
                # (the final band) can be tiny -> short post-in-end tail.
                r0j, Rj = r0s[jump], bands[jump]
                jump_tile = pool.tile([P, 2 * Rj * Ww], f32, name="in_jump",
                                      tag="in_jump", bufs=1)
                rings["scalar"].dma_start(
                    out=jump_tile[:],
                    in_=xi[:, :, r0j * 2 * Ww:(r0j + Rj) * 2 * Ww])
            emit_order = list(range(len(bands)))
            if jump is not None:
                # emit the jump band's compute early (its data lands by
                # ~20us); engines run per program order, so this frees the
                # tail of the ACT/DVE chains for the true last band.
                emit_order.remove(jump)
                emit_order.insert(jump_emit, jump)
            grp_state = {}
            for bi in emit_order:
                r0, R_ = r0s[bi], bands[bi]
                gi, pos, glen = g_of_band[bi]
                n_rows = 2 * R_
                if bi == jump:
                    in_t = jump_tile
                else:
                    in_t = pool.tile([P, n_rows * Ww], f32, name="in_t",
                                     tag="in", bufs=in_bufs)
                    rings[in_ring].dma_start(
                        out=in_t[:],
                        in_=xi[:, :, r0 * 2 * Ww:(r0 + R_) * 2 * Ww])

                # iv[p, rr, w, t]: row rr, col pair w, even/odd col t
                iv = in_t[:].rearrange("p (rr w t) -> p rr w t", rr=n_rows, t=2)
                xe_t = pool.tile([P, n_rows * Wh], bf16, tag="xe", bufs=mid_bufs)
                xo_t = pool.tile([P, n_rows * Wh], bf16, tag="xo", bufs=mid_bufs)
                xev = xe_t[:].rearrange("p (rr j) -> p rr j", rr=n_rows)
                xov = xo_t[:].rearrange("p (rr j) -> p rr j", rr=n_rows)
                nc.scalar.mul(xev[:, :, :], iv[:, :, :, 0], 0.5)
                nc.scalar.mul(xov[:, :, :], iv[:, :, :, 1], 0.5)

                # row butterfly: pair rows (2i, 2i+1), packed bf16
                xep = xe_t[:].rearrange("p (r t j) -> p r t j", t=2, j=Wh)
                xop = xo_t[:].rearrange("p (r t j) -> p r t j", t=2, j=Wh)
                sdt = [pool.tile([P, R_ * Wh], bf16, name=f"sd{i}",
                                 tag=f"sd{i}", bufs=sd_bufs)
                       for i in range(4)]
                se, de, so, do_ = [t[:].rearrange("p (r j) -> p r j", j=Wh)
                                   for t in sdt]
                nc.vector.tensor_add(out=se, in0=xep[:, :, 0, :], in1=xep[:, :, 1, :])
                nc.vector.tensor_sub(out=de, in0=xep[:, :, 0, :], in1=xep[:, :, 1, :])
                nc.vector.tensor_add(out=so, in0=xop[:, :, 0, :], in1=xop[:, :, 1, :])
                nc.vector.tensor_sub(out=do_, in0=xop[:, :, 0, :], in1=xop[:, :, 1, :])

                # col butterfly -> grouped quadrant outputs
                if gi not in grp_state:
                    gsz = sum(bands[bi:bi + glen]) * Wh
                    grp_state[gi] = {
                        "gr0": r0, "goff": 0,
                        "o_ts": [pool.tile([P, gsz], bf16, name=f"o{q}",
                                           tag=f"o{q}", bufs=out_bufs)
                                 for q in range(4)]}
                gs = grp_state[gi]
                o_ts, gr0, goff = gs["o_ts"], gs["gr0"], gs["goff"]
                sl = slice(goff, goff + R_ * Wh)
                ll, lh, hl, hh = [o[:, sl] for o in o_ts]
                nc.vector.tensor_add(out=ll, in0=sdt[0][:], in1=sdt[2][:])
                nc.vector.tensor_add(out=lh, in0=sdt[1][:], in1=sdt[3][:])
                nc.vector.tensor_sub(out=hl, in0=sdt[0][:], in1=sdt[2][:])
                nc.vector.tensor_sub(out=hh, in0=sdt[1][:], in1=sdt[3][:])
                gs["goff"] += R_ * Wh
                if pos == glen - 1:
                    Rg = gs["goff"] // Wh
                    # tail groups: the in-queue (sync/SP HWDGE) is drained
                    # by then — spread out-DMAs over it too
                    qrings = (["sync", "scalar", "gpsimd", "sync"]
                              if gi >= len(groups) - tail_sync_groups
                              else list(out_rings))
                    for q in range(4):
                        rings[qrings[q]].dma_start(
                            out=yo[q][:, :, gr0 * Wh:(gr0 + Rg) * Wh],
                            in_=o_ts[q][:, :Rg * Wh])
    nc.compile()
    return nc


def kernel(x: np.ndarray) -> np.ndarray:
    from concourse.bass_utils import run_bass_kernel_spmd

    x = np.ascontiguousarray(np.asarray(x, dtype=np.float32))
    assert x.shape == (FULL_B, C, H, W), x.shape
    nc = _build_bass()
    shards = np.split(x, N_CORES, axis=0)
    in_maps = [{"x": s} for s in shards]
    res = run_bass_kernel_spmd(nc, in_maps, list(range(N_CORES)))
    return np.concatenate(
        [np.asarray(r["y"]).astype(np.float32) for r in res.results], axis=0)

